# revision 7
# baseline (speedup 1.0000x reference)
"""Multi-head attention kernel for Trainium2, sharded over 8 NeuronCores.

Problem: x[2,2048,1024] -> MHA(16 heads, dh=64) -> out[2,2048,512].

Sharding: core c handles batch b=c//4 and head-group g=c%4 (4 heads each).
Each core computes QKV, attention, and a partial output projection through
its 256-row slice of Wo; host sums the 4 head-group partials and adds bo.

Per-core design (engine budget: ScalarE exp stream is the wall at ~0.83ns
per score element; PE work is cut far below it with fp8 DoubleRow matmuls):
  - QKV projections in fp8 e4m3 DoubleRow (0.5 cyc/row, 2 k-tiles per
    instruction), 3-pass hi/lo error compensation (x*16 and W*256 scaled,
    split into e4m3 hi + e4m3 residual; hh+hl+lh passes, ll dropped).
    Host pre-quantizes, so splitting costs nothing on-chip.
  - Scores S^T[k,q] per head via one DoubleRow matmul per (head, k-tile,
    q-chunk): the pair dim carries Q-hi/Q-lo against 1-pass fp8 K
    (duplicated in SBUF), contraction dh=64. Q8/K8 = 32*(Q|K) quantized
    during the PSUM->SBUF bias-add copies.
  - exp on ScalarE with scale 1/8192 folded in (scores bounded, no max
    subtraction), bf16 output into a 32-slot SBUF ring.
  - AV in natural orientation: attn[q,65] += P^T-tile^T @ V_aug (V has a
    ones column -> row sums land in column 64). bf16, 65-cycle matmuls.
    All 4 q-subtile accumulation groups share one PSUM bank sequentially.
  - normalize: per-partition reciprocal multiply on the PSUM->SBUF copy;
    transpose attn via identity matmul into at^T for the output projection.
  - out partial [s,512] = at^T.T @ Wo (bf16) streamed out per s-tile.

Emission interleaves everything against the exp stream: scores for unit
(qc,h) + AV of the previous unit per k-pair, Q-projection and out-proj
fillers in fixed slots, so ScalarE never starves after the DMA lead-in.
"""

import sys

sys.path.insert(0, "/opt/trn_rl_repo")

import numpy as np
from contextlib import ExitStack

# Problem shapes (hardcoded per the harness contract).
B = 2
S = 2048
DIN = 1024
H = 16
DH = 64
DMODEL = H * DH  # 1024
DOUT = 512
NCORES = 8

# Per-core shard shapes.
HPC = 4  # heads per core
DQ = HPC * DH  # 256: per-core QKV width
KT = DIN // 128  # 8  k-tiles over d_in
MT = DQ // 128  # 2  m-tiles over per-core dq
QC = S // 512  # 4  q-chunks of 512
SKT = S // 128  # 16 seq k-tiles
VW = DH + 1  # 65: V columns per head incl. ones column
ESL = 32  # et ring slots

SX = 16.0  # x fp8 scale
SW = 256.0  # W fp8 scale
SQ = 32.0  # Q/K fp8 scale
PSC = 1.0 / (SX * SW)  # proj psum -> true value
QSC = SQ * PSC  # proj psum -> q8/k8 value (2^-7)
ESC = 1.0 / (SQ * SQ * np.sqrt(DH))  # score psum -> exp argument


def build_program(repeat=1):
    from concourse import bacc, tile
    import concourse.bass as bass
    import concourse.mybir as mybir

    f32 = mybir.dt.float32
    bf16 = mybir.dt.bfloat16
    f8 = mybir.dt.float8e4
    Exp = mybir.ActivationFunctionType.Exp
    DR = mybir.MatmulPerfMode.DoubleRow
    mult = mybir.AluOpType.mult
    add = mybir.AluOpType.add
    sub = mybir.AluOpType.subtract

    nc = bacc.Bacc("TRN2", target_bir_lowering=False, debug=False)

    xh_d = nc.dram_tensor("xh", [QC, 128, KT, 512], f8, kind="ExternalInput")
    xl_d = nc.dram_tensor("xl", [QC, 128, KT, 512], f8, kind="ExternalInput")
    wqh_d = nc.dram_tensor("wqh", [128, KT, DQ], f8, kind="ExternalInput")
    wql_d = nc.dram_tensor("wql", [128, KT, DQ], f8, kind="ExternalInput")
    wkh_d = nc.dram_tensor("wkh", [128, KT, DQ], f8, kind="ExternalInput")
    wkl_d = nc.dram_tensor("wkl", [128, KT, DQ], f8, kind="ExternalInput")
    wvh_d = nc.dram_tensor("wvh", [128, KT, DQ], f8, kind="ExternalInput")
    wvl_d = nc.dram_tensor("wvl", [128, KT, DQ], f8, kind="ExternalInput")
    bq_d = nc.dram_tensor("bq", [128, MT], f32, kind="ExternalInput")
    bk_d = nc.dram_tensor("bk", [128, MT], f32, kind="ExternalInput")
    bv_d = nc.dram_tensor("bv", [128, HPC, DH], bf16, kind="ExternalInput")
    wo_d = nc.dram_tensor("wo", [128, MT, DOUT], bf16, kind="ExternalInput")
    id_d = nc.dram_tensor("ident", [128, 128], bf16, kind="ExternalInput")
    out_d = nc.dram_tensor("out", [S, DOUT], f32, kind="ExternalOutput")

    with tile.TileContext(nc) as tc, ExitStack() as octx:
        consts = octx.enter_context(tc.tile_pool(name="consts", bufs=1))
        id16 = consts.tile([128, 128], bf16)
        bq32 = consts.tile([128, MT], f32)
        bk32 = consts.tile([128, MT], f32)
        bvb = consts.tile([128, HPC, DH], bf16)
        wo16 = consts.tile([128, MT, DOUT], bf16)
        nc.sync.dma_start(id16[:], id_d[:])
        nc.sync.dma_start(bq32[:], bq_d[:])
        nc.sync.dma_start(bk32[:], bk_d[:])
        nc.sync.dma_start(bvb[:], bv_d[:])
        nc.sync.dma_start(wo16[:], wo_d[:])

        for _rep in range(repeat):
            with ExitStack() as rctx:
                keep = rctx.enter_context(tc.tile_pool(name="keep", bufs=1))
                xh_sb = keep.tile([128, KT, S], f8)
                xl_sb = keep.tile([128, KT, S], f8)
                wqh = keep.tile([128, KT, DQ], f8)
                wql = keep.tile([128, KT, DQ], f8)
                wkh = keep.tile([128, KT, DQ], f8)
                wkl = keep.tile([128, KT, DQ], f8)
                wvh = keep.tile([128, KT, DQ], f8)
                wvl = keep.tile([128, KT, DQ], f8)
                # Q8/K8: head h=2m+j at partitions 64j..64j+64, m-tile m;
                # dim2 = (hi,lo) for Q8, duplicate slots for K8.
                q8_sb = keep.tile([128, MT, 2, S], f8)
                k8_sb = keep.tile([128, MT, 2, S], f8)
                v_sb = keep.tile([128, SKT, HPC, VW], bf16)
                et_sb = keep.tile([128, ESL, 512], bf16)
                at_sb = keep.tile([128, MT, S], bf16)
                nc.vector.memset(v_sb[:, :, :, DH], 1.0)

                sc_ps = rctx.enter_context(
                    tc.tile_pool(name="sc_ps", bufs=2, space="PSUM")
                )
                pj_ps = rctx.enter_context(
                    tc.tile_pool(name="pj_ps", bufs=2, space="PSUM")
                )
                av_ps = rctx.enter_context(
                    tc.tile_pool(name="av_ps", bufs=2, space="PSUM")
                )
                sm = rctx.enter_context(tc.tile_pool(name="sm", bufs=2))

                def dr12(ps, lhs_hl, rhs_hl):
                    """12 DoubleRow matmuls: 3-pass hi/lo over 4 k-tile pairs.

                    lhs_hl(sl, hi) / rhs_hl(sl, hi) -> stationary/moving
                    slices for k-tile pair sl; passes hh + hl + lh."""
                    for tp in range(KT // 2):
                        sl = slice(2 * tp, 2 * tp + 2)
                        first, last = tp == 0, tp == KT // 2 - 1
                        nc.tensor.matmul(
                            ps, lhs_hl(sl, True), rhs_hl(sl, True),
                            start=first, stop=False, perf_mode=DR,
                        )
                        nc.tensor.matmul(
                            ps, lhs_hl(sl, True), rhs_hl(sl, False),
                            start=False, stop=False, perf_mode=DR,
                        )
                        nc.tensor.matmul(
                            ps, lhs_hl(sl, False), rhs_hl(sl, True),
                            start=False, stop=last, perf_mode=DR,
                        )

                def qk_proj(wh, wl, m, qc, is_q):
                    """Q^T/K^T m-tile for q-chunk qc -> q8/k8 (scaled fp8)."""
                    qsl = slice(qc * 512, (qc + 1) * 512)
                    msl = slice(m * 128, (m + 1) * 128)
                    ps = pj_ps.tile([128, 512], f32, tag="pj", name="ps")
                    dr12(
                        ps[:],
                        lambda sl, hi: (wh if hi else wl)[:, sl, msl],
                        lambda sl, hi: (xh_sb if hi else xl_sb)[:, sl, qsl],
                    )
                    with nc.allow_low_precision(reason="fp8 by design"):
                        if is_q:
                            q16 = sm.tile([128, 512], bf16, tag="q16")
                            nc.vector.tensor_scalar(
                                q16[:], ps[:], QSC, bq32[:, m : m + 1], mult, add
                            )
                            nc.vector.tensor_copy(
                                q8_sb[:, m, 0, qsl], q16[:]
                            )
                            nc.vector.tensor_tensor(
                                q8_sb[:, m, 1, qsl],
                                q16[:],
                                q8_sb[:, m, 0, qsl],
                                sub,
                            )
                        else:
                            nc.vector.tensor_scalar(
                                k8_sb[:, m, 0, qsl],
                                ps[:],
                                QSC,
                                bk32[:, m : m + 1],
                                mult,
                                add,
                            )
                            nc.vector.tensor_copy(
                                k8_sb[:, m, 1, qsl], k8_sb[:, m, 0, qsl]
                            )

                def v_proj(st):
                    """Natural-orientation V s-tile st -> v_sb (bf16 + bias)."""
                    ssl = slice(st * 128, (st + 1) * 128)
                    ps = pj_ps.tile([128, 512], f32, tag="pj", name="ps")
                    dr12(
                        ps[:, :DQ],
                        lambda sl, hi: (xh_sb if hi else xl_sb)[:, sl, ssl],
                        lambda sl, hi: (wvh if hi else wvl)[:, sl, :],
                    )
                    with nc.allow_low_precision(reason="bf16 by design"):
                        nc.vector.scalar_tensor_tensor(
                            v_sb[:, st, :, :DH],
                            ps[:, :DQ].rearrange("p (h d) -> p h d", h=HPC),
                            PSC,
                            bvb[:],
                            mult,
                            add,
                        )

                def scores_unit(qc, h, u):
                    """k-tiles 2u,2u+1 of S^T for (qc,h): 2 DR matmuls + exp."""
                    j, m = h % 2, h // 2
                    base = slice(64 * j, 64 * j + 64)
                    qsl = slice(qc * 512, (qc + 1) * 512)
                    sc = sc_ps.tile([128, 2, 512], f32, tag="sc", name="sc")
                    for i in range(2):
                        kt = 2 * u + i
                        nc.tensor.matmul(
                            sc[:, i, :],
                            k8_sb[base, m, :, kt * 128 : (kt + 1) * 128],
                            q8_sb[base, m, :, qsl],
                            start=True,
                            stop=True,
                            perf_mode=DR,
                        )
                    slot = ((qc * HPC + h) * SKT + 2 * u) % ESL
                    with nc.allow_low_precision(reason="bf16 probs by design"):
                        nc.scalar.activation(
                            et_sb[:, slot : slot + 2, :], sc[:], Exp, scale=ESC
                        )

                def av_slice(qc, h, av3, qt, k0):
                    """8 k-tiles of the attn accumulation for q-subtile qt."""
                    ubase = (qc * HPC + h) * SKT
                    qts = slice(qt * 128, (qt + 1) * 128)
                    for kt in range(k0, k0 + 8):
                        nc.tensor.matmul(
                            av3[:, qt, :],
                            et_sb[:, (ubase + kt) % ESL, qts],
                            v_sb[:, kt, h, :],
                            start=(kt == 0),
                            stop=(kt == SKT - 1),
                        )

                def finish(qc, h, av, av3, a16):
                    """Normalize closed attn accums; transpose on pair end."""
                    j = h % 2
                    rec = sm.tile([128, HPC], f32, tag="rec")
                    with nc.allow_low_precision(reason="recip of ~2e3 sums"):
                        nc.vector.reciprocal(rec[:], av3[:, :, DH])
                        for qt in range(4):
                            nc.vector.tensor_scalar(
                                a16[:, qt, j, :],
                                av3[:, qt, :DH],
                                rec[:, qt : qt + 1],
                                None,
                                mult,
                            )
                    if j == 1:
                        p = h // 2
                        tp = av[:, HPC * VW : HPC * VW + 128]
                        for qt in range(4):
                            nc.tensor.matmul(
                                tp,
                                a16[:, qt, :, :].rearrange("p a b -> p (a b)"),
                                id16[:],
                                start=True,
                                stop=True,
                            )
                            with nc.allow_low_precision(reason="bf16 attn"):
                                nc.vector.tensor_copy(
                                    at_sb[
                                        :,
                                        p,
                                        qc * 512 + qt * 128 : qc * 512 + qt * 128 + 128,
                                    ],
                                    tp,
                                )

                def out_proj(m):
                    """Output partial for s-tile m."""
                    ps = pj_ps.tile([128, 512], f32, tag="pj", name="ps")
                    for k2 in range(MT):
                        nc.tensor.matmul(
                            ps[:],
                            at_sb[:, k2, m * 128 : (m + 1) * 128],
                            wo16[:, k2, :],
                            start=(k2 == 0),
                            stop=(k2 == MT - 1),
                        )
                    ot = sm.tile([128, DOUT], f32, tag="ot")
                    nc.vector.tensor_copy(ot[:], ps[:])
                    nc.sync.dma_start(out_d[m * 128 : (m + 1) * 128, :], ot[:])

                # ---- Lead-in: PE pstate warmup on the identity while DMA
                # streams x by k-chunk; project K-m0 (all chunks), Q-m0
                # (chunk 0) and V; run (0,0) score units as chunks land.
                # K-m1 / Q-m1 projections are deferred into pipeline fillers.
                wm = sc_ps.tile([128, 2, 512], f32, tag="sc", name="wm")
                for _ in range(24):
                    nc.tensor.matmul(
                        wm[:, 0, :128], id16[:], id16[:], start=True, stop=True
                    )
                nc.sync.dma_start(wkh[:], wkh_d[:])
                nc.sync.dma_start(wkl[:], wkl_d[:])
                for kc in range(QC):
                    csl = slice(kc * 512, (kc + 1) * 512)
                    nc.sync.dma_start(xh_sb[:, :, csl], xh_d[kc])
                    nc.sync.dma_start(xl_sb[:, :, csl], xl_d[kc])
                    if kc == 0:
                        nc.sync.dma_start(wqh[:], wqh_d[:])
                        nc.sync.dma_start(wql[:], wql_d[:])
                        nc.sync.dma_start(wvh[:], wvh_d[:])
                        nc.sync.dma_start(wvl[:], wvl_d[:])
                    qk_proj(wkh, wkl, 0, kc, is_q=False)
                    if kc == 0:
                        qk_proj(wqh, wql, 0, 0, is_q=True)
                    scores_unit(0, 0, 2 * kc)
                    scores_unit(0, 0, 2 * kc + 1)
                    for st in range(4 * kc, 4 * kc + 4):
                        v_proj(st)

                # ---- Main pipeline over (qc, h) units.
                units = [(qc, h) for qc in range(QC) for h in range(HPC)]
                prev = None
                for qc, h in units:
                    av = av_ps.tile([128, HPC * VW + 128], f32, tag="av", name="av")
                    av3 = av[:, : HPC * VW].rearrange("p (t c) -> p t c", t=HPC)
                    if h % 2 == 0:
                        a16 = sm.tile([128, 4, 2, DH], bf16, tag="a16", bufs=2)
                    cur = (qc, h, av, av3, a16)
                    for u in range(8):
                        if (qc, h) != (0, 0):
                            scores_unit(qc, h, u)
                        if prev is not None:
                            av_slice(prev[0], prev[1], prev[3], u // 2, 8 * (u % 2))
                        if (qc, h) == (0, 0) and u % 2 == 1:
                            qk_proj(wkh, wkl, 1, u // 2, is_q=False)
                        if (qc, h) == (0, 0) and u == 6:
                            qk_proj(wqh, wql, 1, 0, is_q=True)
                        if u == 2 and h < MT and qc < QC - 1:
                            qk_proj(wqh, wql, h, qc + 1, is_q=True)
                        if qc >= 1 and u == 4 and h >= 1:
                            out_proj(4 * (qc - 1) + h - 1)
                        if qc >= 1 and u == 6 and h == 3:
                            out_proj(4 * (qc - 1) + 3)
                    if prev is not None:
                        finish(prev[0], prev[1], prev[2], prev[3], prev[4])
                    prev = cur

                # ---- Tail: close the last unit (3,3) pipelined per
                # q-subtile: out-proj m-tile 12+qt only needs subtile qt.
                qc, h, av, av3, a16 = prev
                tp = av[:, HPC * VW : HPC * VW + 128]
                rec = sm.tile([128, HPC], f32, tag="rec")
                for qt in range(4):
                    av_slice(qc, h, av3, qt, 0)
                    av_slice(qc, h, av3, qt, 8)
                    with nc.allow_low_precision(reason="recip of ~2e3 sums"):
                        nc.vector.reciprocal(
                            rec[:, qt : qt + 1], av3[:, qt, DH : DH + 1]
                        )
                        nc.vector.tensor_scalar(
                            a16[:, qt, 1, :],
                            av3[:, qt, :DH],
                            rec[:, qt : qt + 1],
                            None,
                            mult,
                        )
                    nc.tensor.matmul(
                        tp,
                        a16[:, qt, :, :].rearrange("p a b -> p (a b)"),
                        id16[:],
                        start=True,
                        stop=True,
                    )
                    with nc.allow_low_precision(reason="bf16 attn"):
                        nc.vector.tensor_copy(
                            at_sb[
                                :,
                                1,
                                qc * 512 + qt * 128 : qc * 512 + qt * 128 + 128,
                            ],
                            tp,
                        )
                    out_proj(4 * (QC - 1) + qt)

    nc.compile()
    return nc


def shard_inputs(inputs):
    """Build the 8 per-core input maps: core c -> batch c//4, head-group c%4."""
    import ml_dtypes

    f8 = ml_dtypes.float8_e4m3
    bf = ml_dtypes.bfloat16

    x = np.asarray(inputs["x"], dtype=np.float32)
    Wq = np.asarray(inputs["Wq"], dtype=np.float32)
    Wk = np.asarray(inputs["Wk"], dtype=np.float32)
    Wv = np.asarray(inputs["Wv"], dtype=np.float32)
    bq = np.asarray(inputs["bq"], dtype=np.float32)
    bk = np.asarray(inputs["bk"], dtype=np.float32)
    bv = np.asarray(inputs["bv"], dtype=np.float32)
    Wo = np.asarray(inputs["Wo"], dtype=np.float32)

    def hilo(a, scale):
        s = (a * scale).astype(np.float32)
        hi = s.astype(f8)
        lo = (s - hi.astype(np.float32)).astype(f8)
        return hi, lo

    def xprep(xb):
        # [S, DIN] -> x^T [128, KT, S] -> DMA layout [QC, 128, KT, 512]
        xt = xb.T.reshape(KT, 128, QC, 512).transpose(2, 1, 0, 3)
        return np.ascontiguousarray(xt)

    def wprep(W, g):
        w = W[:, g * DQ : (g + 1) * DQ]  # [1024, 256]
        return np.ascontiguousarray(w.reshape(KT, 128, DQ).transpose(1, 0, 2))

    ident = np.eye(128, dtype=np.float32).astype(bf)

    in_maps = []
    for c in range(NCORES):
        b, g = divmod(c, HPC)
        xh, xl = hilo(xprep(x[b]), SX)
        m = {"xh": xh, "xl": xl, "ident": ident}
        for nm, W in (("wq", Wq), ("wk", Wk), ("wv", Wv)):
            hi, lo = hilo(wprep(W, g), SW)
            m[nm + "h"], m[nm + "l"] = hi, lo
        bqg = bq[g * DQ : (g + 1) * DQ] * SQ
        bkg = bk[g * DQ : (g + 1) * DQ] * SQ
        m["bq"] = np.ascontiguousarray(bqg.reshape(MT, 128).T)
        m["bk"] = np.ascontiguousarray(bkg.reshape(MT, 128).T)
        bvg = bv[g * DQ : (g + 1) * DQ].reshape(HPC, DH)
        m["bv"] = np.broadcast_to(bvg, (128, HPC, DH)).astype(bf)
        wog = Wo[g * DQ : (g + 1) * DQ, :]
        m["wo"] = (
            wog.reshape(MT, 128, DOUT).transpose(1, 0, 2).astype(bf)
        )
        m["wo"] = np.ascontiguousarray(m["wo"])
        in_maps.append(m)
    return in_maps


_PROGRAM_CACHE = []


def run_on_hw(inputs, trace=False):
    from concourse.bass_utils import run_bass_kernel_spmd

    if not _PROGRAM_CACHE:
        _PROGRAM_CACHE.append(build_program(1))
    nc = _PROGRAM_CACHE[0]
    in_maps = shard_inputs(inputs)
    res = run_bass_kernel_spmd(nc, in_maps, list(range(NCORES)), trace=False)
    bo = np.asarray(inputs["bo"], dtype=np.float32)
    out = np.zeros((B, S, DOUT), dtype=np.float32)
    for c in range(NCORES):
        out[c // HPC] += res.results[c]["out"]
    out += bo
    return out, res


def kernel(**inputs):
    out, _ = run_on_hw(inputs, trace=False)
    return out


# revision 10
# speedup vs baseline: 1.0717x; 1.0717x over previous
"""Multi-head attention kernel for Trainium2, sharded over 8 NeuronCores.

Problem: x[2,2048,1024] -> MHA(16 heads, dh=64) -> out[2,2048,512].

Sharding: core c handles batch b=c//4 and head-group g=c%4 (4 heads each).
Each core computes QKV, attention, and a partial output projection through
its 256-row slice of Wo; host sums the 4 head-group partials and adds bo.

Per-core design (engine budget: ScalarE exp stream is the wall at ~0.83ns
per score element; PE work is cut far below it with fp8 DoubleRow matmuls):
  - QKV projections in fp8 e4m3 DoubleRow (0.5 cyc/row, 2 k-tiles per
    instruction), 3-pass hi/lo error compensation (x*16 and W*256 scaled,
    split into e4m3 hi + e4m3 residual; hh+hl+lh passes, ll dropped).
    Host pre-quantizes, so splitting costs nothing on-chip.
  - Scores S^T[k,q] per head via one DoubleRow matmul per (head, k-tile,
    q-chunk): the pair dim carries Q-hi/Q-lo against 1-pass fp8 K
    (duplicated in SBUF), contraction dh=64. Q8/K8 = 32*(Q|K) quantized
    during the PSUM->SBUF bias-add copies.
  - exp on ScalarE with scale 1/8192 folded in (scores bounded, no max
    subtraction), bf16 output into a 32-slot SBUF ring.
  - AV in natural orientation: attn[q,65] += P^T-tile^T @ V_aug (V has a
    ones column -> row sums land in column 64). bf16, 65-cycle matmuls.
    All 4 q-subtile accumulation groups share one PSUM bank sequentially.
  - normalize: per-partition reciprocal multiply on the PSUM->SBUF copy;
    transpose attn via identity matmul into at^T for the output projection.
  - out partial [s,512] = at^T.T @ Wo (bf16) streamed out per s-tile.

Emission interleaves everything against the exp stream: scores for unit
(qc,h) + AV of the previous unit per k-pair, Q-projection and out-proj
fillers in fixed slots, so ScalarE never starves after the DMA lead-in.
"""

import sys

sys.path.insert(0, "/opt/trn_rl_repo")

import numpy as np
from contextlib import ExitStack

# Problem shapes (hardcoded per the harness contract).
B = 2
S = 2048
DIN = 1024
H = 16
DH = 64
DMODEL = H * DH  # 1024
DOUT = 512
NCORES = 8

# Per-core shard shapes.
HPC = 4  # heads per core
DQ = HPC * DH  # 256: per-core QKV width
KT = DIN // 128  # 8  k-tiles over d_in
MT = DQ // 128  # 2  m-tiles over per-core dq
QC = S // 512  # 4  q-chunks of 512
SKT = S // 128  # 16 seq k-tiles
VW = DH + 1  # 65: V columns per head incl. ones column
ESL = 32  # et ring slots

SX = 16.0  # x fp8 scale
SW = 256.0  # W fp8 scale
SQ = 32.0  # Q/K fp8 scale
PSC = 1.0 / (SX * SW)  # proj psum -> true value
QSC = SQ * PSC  # proj psum -> q8/k8 value (2^-7)
ESC = 1.0 / (SQ * SQ * np.sqrt(DH))  # score psum -> exp argument


def build_program(repeat=1):
    from concourse import bacc, tile
    import concourse.bass as bass
    import concourse.mybir as mybir

    f32 = mybir.dt.float32
    bf16 = mybir.dt.bfloat16
    f8 = mybir.dt.float8e4
    Exp = mybir.ActivationFunctionType.Exp
    DR = mybir.MatmulPerfMode.DoubleRow
    mult = mybir.AluOpType.mult
    add = mybir.AluOpType.add
    sub = mybir.AluOpType.subtract

    nc = bacc.Bacc("TRN2", target_bir_lowering=False, debug=False)

    xh_d = nc.dram_tensor("xh", [QC, 128, KT, 512], f8, kind="ExternalInput")
    xl_d = nc.dram_tensor("xl", [QC, 128, KT, 512], f8, kind="ExternalInput")
    wqh_d = nc.dram_tensor("wqh", [128, KT, DQ], f8, kind="ExternalInput")
    wql_d = nc.dram_tensor("wql", [128, KT, DQ], f8, kind="ExternalInput")
    wkh_d = nc.dram_tensor("wkh", [128, KT, DQ], f8, kind="ExternalInput")
    wkl_d = nc.dram_tensor("wkl", [128, KT, DQ], f8, kind="ExternalInput")
    wvh_d = nc.dram_tensor("wvh", [128, KT, DQ], f8, kind="ExternalInput")
    wvl_d = nc.dram_tensor("wvl", [128, KT, DQ], f8, kind="ExternalInput")
    bq_d = nc.dram_tensor("bq", [128, MT], f32, kind="ExternalInput")
    bk_d = nc.dram_tensor("bk", [128, MT], f32, kind="ExternalInput")
    bv_d = nc.dram_tensor("bv", [128, HPC, DH], bf16, kind="ExternalInput")
    wo_d = nc.dram_tensor("wo", [128, MT, DOUT], bf16, kind="ExternalInput")
    id_d = nc.dram_tensor("ident", [128, 128], bf16, kind="ExternalInput")
    out_d = nc.dram_tensor("out", [S, DOUT], f32, kind="ExternalOutput")

    with tile.TileContext(nc) as tc, ExitStack() as octx:
        consts = octx.enter_context(tc.tile_pool(name="consts", bufs=1))
        id16 = consts.tile([128, 128], bf16)
        bq32 = consts.tile([128, MT], f32)
        bk32 = consts.tile([128, MT], f32)
        bvb = consts.tile([128, HPC, DH], bf16)
        wo16 = consts.tile([128, MT, DOUT], bf16)
        nc.sync.dma_start(id16[:], id_d[:])

        for _rep in range(repeat):
            with ExitStack() as rctx:
                keep = rctx.enter_context(tc.tile_pool(name="keep", bufs=1))
                xh_sb = keep.tile([128, KT, S], f8)
                xl_sb = keep.tile([128, KT, S], f8)
                wqh = keep.tile([128, KT, DQ], f8)
                wql = keep.tile([128, KT, DQ], f8)
                wkh = keep.tile([128, KT, DQ], f8)
                wkl = keep.tile([128, KT, DQ], f8)
                wvh = keep.tile([128, KT, DQ], f8)
                wvl = keep.tile([128, KT, DQ], f8)
                # Q8/K8: head h=2m+j at partitions 64j..64j+64, m-tile m;
                # dim2 = (hi,lo) for Q8, duplicate slots for K8.
                q8_sb = keep.tile([128, MT, 2, S], f8)
                k8_sb = keep.tile([128, MT, 2, S], f8)
                v_sb = keep.tile([128, SKT, HPC, VW], bf16)
                et_sb = keep.tile([128, ESL, 512], bf16)
                at_sb = keep.tile([128, MT, S], bf16)
                nc.vector.memset(v_sb[:, :, :, DH], 1.0)

                sc_ps = rctx.enter_context(
                    tc.tile_pool(name="sc_ps", bufs=2, space="PSUM")
                )
                pj_ps = rctx.enter_context(
                    tc.tile_pool(name="pj_ps", bufs=2, space="PSUM")
                )
                av_ps = rctx.enter_context(
                    tc.tile_pool(name="av_ps", bufs=2, space="PSUM")
                )
                sm = rctx.enter_context(tc.tile_pool(name="sm", bufs=2))

                def dr12(ps, lhs_hl, rhs_hl):
                    """12 DoubleRow matmuls: 3-pass hi/lo over 4 k-tile pairs.

                    lhs_hl(sl, hi) / rhs_hl(sl, hi) -> stationary/moving
                    slices for k-tile pair sl; passes hh + hl + lh."""
                    for tp in range(KT // 2):
                        sl = slice(2 * tp, 2 * tp + 2)
                        first, last = tp == 0, tp == KT // 2 - 1
                        nc.tensor.matmul(
                            ps, lhs_hl(sl, True), rhs_hl(sl, True),
                            start=first, stop=False, perf_mode=DR,
                        )
                        nc.tensor.matmul(
                            ps, lhs_hl(sl, True), rhs_hl(sl, False),
                            start=False, stop=False, perf_mode=DR,
                        )
                        nc.tensor.matmul(
                            ps, lhs_hl(sl, False), rhs_hl(sl, True),
                            start=False, stop=last, perf_mode=DR,
                        )

                def qk_proj(wh, wl, m, qc, is_q):
                    """Q^T/K^T m-tile for q-chunk qc -> q8/k8 (scaled fp8)."""
                    qsl = slice(qc * 512, (qc + 1) * 512)
                    msl = slice(m * 128, (m + 1) * 128)
                    ps = pj_ps.tile([128, 512], f32, tag="pj", name="ps")
                    dr12(
                        ps[:],
                        lambda sl, hi: (wh if hi else wl)[:, sl, msl],
                        lambda sl, hi: (xh_sb if hi else xl_sb)[:, sl, qsl],
                    )
                    with nc.allow_low_precision(reason="fp8 by design"):
                        if is_q:
                            q16 = sm.tile([128, 512], bf16, tag="q16")
                            nc.vector.tensor_scalar(
                                q16[:], ps[:], QSC, bq32[:, m : m + 1], mult, add
                            )
                            nc.vector.tensor_copy(
                                q8_sb[:, m, 0, qsl], q16[:]
                            )
                            nc.vector.tensor_tensor(
                                q8_sb[:, m, 1, qsl],
                                q16[:],
                                q8_sb[:, m, 0, qsl],
                                sub,
                            )
                        else:
                            nc.vector.tensor_scalar(
                                k8_sb[:, m, 0, qsl],
                                ps[:],
                                QSC,
                                bk32[:, m : m + 1],
                                mult,
                                add,
                            )
                            nc.vector.tensor_copy(
                                k8_sb[:, m, 1, qsl], k8_sb[:, m, 0, qsl]
                            )

                def v_proj(st):
                    """Natural-orientation V s-tile st -> v_sb (bf16 + bias)."""
                    ssl = slice(st * 128, (st + 1) * 128)
                    ps = pj_ps.tile([128, 512], f32, tag="pj", name="ps")
                    dr12(
                        ps[:, :DQ],
                        lambda sl, hi: (xh_sb if hi else xl_sb)[:, sl, ssl],
                        lambda sl, hi: (wvh if hi else wvl)[:, sl, :],
                    )
                    with nc.allow_low_precision(reason="bf16 by design"):
                        nc.vector.scalar_tensor_tensor(
                            v_sb[:, st, :, :DH],
                            ps[:, :DQ].rearrange("p (h d) -> p h d", h=HPC),
                            PSC,
                            bvb[:],
                            mult,
                            add,
                        )

                def scores_unit(qc, h, u):
                    """k-tiles 2u,2u+1 of S^T for (qc,h): 2 DR matmuls + exp."""
                    j, m = h % 2, h // 2
                    base = slice(64 * j, 64 * j + 64)
                    qsl = slice(qc * 512, (qc + 1) * 512)
                    sc = sc_ps.tile([128, 2, 512], f32, tag="sc", name="sc")
                    for i in range(2):
                        kt = 2 * u + i
                        nc.tensor.matmul(
                            sc[:, i, :],
                            k8_sb[base, m, :, kt * 128 : (kt + 1) * 128],
                            q8_sb[base, m, :, qsl],
                            start=True,
                            stop=True,
                            perf_mode=DR,
                        )
                    slot = ((qc * HPC + h) * SKT + 2 * u) % ESL
                    with nc.allow_low_precision(reason="bf16 probs by design"):
                        nc.scalar.activation(
                            et_sb[:, slot : slot + 2, :], sc[:], Exp, scale=ESC
                        )

                def av_slice(qc, h, av3, qt, k0):
                    """8 k-tiles of the attn accumulation for q-subtile qt."""
                    ubase = (qc * HPC + h) * SKT
                    qts = slice(qt * 128, (qt + 1) * 128)
                    for kt in range(k0, k0 + 8):
                        nc.tensor.matmul(
                            av3[:, qt, :],
                            et_sb[:, (ubase + kt) % ESL, qts],
                            v_sb[:, kt, h, :],
                            start=(kt == 0),
                            stop=(kt == SKT - 1),
                        )

                def finish(qc, h, av, av3, a16):
                    """Normalize closed attn accums; transpose on pair end."""
                    j = h % 2
                    rec = sm.tile([128, HPC], f32, tag="rec")
                    with nc.allow_low_precision(reason="recip of ~2e3 sums"):
                        nc.vector.reciprocal(rec[:], av3[:, :, DH])
                        for qt in range(4):
                            nc.vector.tensor_scalar(
                                a16[:, qt, j, :],
                                av3[:, qt, :DH],
                                rec[:, qt : qt + 1],
                                None,
                                mult,
                            )
                    if j == 1:
                        p = h // 2
                        tp = av[:, HPC * VW : HPC * VW + 128]
                        for qt in range(4):
                            nc.tensor.matmul(
                                tp,
                                a16[:, qt, :, :].rearrange("p a b -> p (a b)"),
                                id16[:],
                                start=True,
                                stop=True,
                            )
                            with nc.allow_low_precision(reason="bf16 attn"):
                                nc.vector.tensor_copy(
                                    at_sb[
                                        :,
                                        p,
                                        qc * 512 + qt * 128 : qc * 512 + qt * 128 + 128,
                                    ],
                                    tp,
                                )

                def out_proj(m):
                    """Output partial for s-tile m."""
                    ps = pj_ps.tile([128, 512], f32, tag="pj", name="ps")
                    for k2 in range(MT):
                        nc.tensor.matmul(
                            ps[:],
                            at_sb[:, k2, m * 128 : (m + 1) * 128],
                            wo16[:, k2, :],
                            start=(k2 == 0),
                            stop=(k2 == MT - 1),
                        )
                    ot = sm.tile([128, DOUT], f32, tag="ot")
                    nc.vector.tensor_copy(ot[:], ps[:])
                    nc.sync.dma_start(out_d[m * 128 : (m + 1) * 128, :], ot[:])

                # ---- Lead-in. DMA issue is serialized per engine, so x-hi
                # chunks go out on the idle Pool engine while SP carries x-lo
                # interleaved with weights; late-needed consts trail. PE
                # warms its pstate on the identity, then Q-projection first
                # (it heads the longest DVE chain), K, and the (0,0) score
                # units per chunk. V and m1 projections defer to fillers.
                nc.sync.dma_start(wqh[:], wqh_d[:])
                nc.sync.dma_start(wql[:], wql_d[:])
                nc.gpsimd.dma_start(xh_sb[:, :, 0:512], xh_d[0])
                nc.gpsimd.dma_start(bq32[:], bq_d[:])
                nc.gpsimd.dma_start(bk32[:], bk_d[:])
                nc.gpsimd.dma_start(wkh[:], wkh_d[:])
                nc.gpsimd.dma_start(wkl[:], wkl_d[:])
                nc.sync.dma_start(xl_sb[:, :, 0:512], xl_d[0])
                for kc in range(1, QC):
                    csl = slice(kc * 512, (kc + 1) * 512)
                    nc.gpsimd.dma_start(xh_sb[:, :, csl], xh_d[kc])
                    nc.sync.dma_start(xl_sb[:, :, csl], xl_d[kc])
                    if kc == 1:
                        nc.sync.dma_start(wvh[:], wvh_d[:])
                        nc.sync.dma_start(wvl[:], wvl_d[:])
                nc.sync.dma_start(bvb[:], bv_d[:])
                nc.sync.dma_start(wo16[:], wo_d[:])
                wm = sc_ps.tile([128, 2, 512], f32, tag="sc", name="wm")
                for _ in range(28):
                    nc.tensor.matmul(
                        wm[:, 0, :128], id16[:], id16[:], start=True, stop=True
                    )
                qk_proj(wqh, wql, 0, 0, is_q=True)
                for kc in range(QC):
                    qk_proj(wkh, wkl, 0, kc, is_q=False)
                    scores_unit(0, 0, 2 * kc)
                    scores_unit(0, 0, 2 * kc + 1)

                # ---- Main pipeline over (qc, h) units.
                units = [(qc, h) for qc in range(QC) for h in range(HPC)]
                prev = None
                for qc, h in units:
                    av = av_ps.tile([128, HPC * VW + 128], f32, tag="av", name="av")
                    av3 = av[:, : HPC * VW].rearrange("p (t c) -> p t c", t=HPC)
                    if h % 2 == 0:
                        a16 = sm.tile([128, 4, 2, DH], bf16, tag="a16", bufs=2)
                    cur = (qc, h, av, av3, a16)
                    for u in range(8):
                        if (qc, h) != (0, 0):
                            scores_unit(qc, h, u)
                        if prev is not None:
                            av_slice(prev[0], prev[1], prev[3], u // 2, 8 * (u % 2))
                        if (qc, h) == (0, 0):
                            for st in (2 * u, 2 * u + 1):
                                if st < 14:
                                    v_proj(st)
                        if (qc, h) == (0, 1):
                            if u == 0:
                                v_proj(14)
                                v_proj(15)
                            if u % 2 == 1:
                                qk_proj(wkh, wkl, 1, u // 2, is_q=False)
                            if u == 6:
                                qk_proj(wqh, wql, 1, 0, is_q=True)
                        if u == 2 and h >= 2 and qc < QC - 1:
                            qk_proj(wqh, wql, h - 2, qc + 1, is_q=True)
                        if qc >= 1 and u == 4 and h >= 1:
                            out_proj(4 * (qc - 1) + h - 1)
                        if qc >= 1 and u == 6 and h == 3:
                            out_proj(4 * (qc - 1) + 3)
                    if prev is not None:
                        finish(prev[0], prev[1], prev[2], prev[3], prev[4])
                    prev = cur

                # ---- Tail: close the last unit (3,3) pipelined per
                # q-subtile: out-proj m-tile 12+qt only needs subtile qt.
                qc, h, av, av3, a16 = prev
                tp = av[:, HPC * VW : HPC * VW + 128]
                rec = sm.tile([128, HPC], f32, tag="rec")
                for qt in range(4):
                    av_slice(qc, h, av3, qt, 0)
                    av_slice(qc, h, av3, qt, 8)
                    with nc.allow_low_precision(reason="recip of ~2e3 sums"):
                        nc.vector.reciprocal(
                            rec[:, qt : qt + 1], av3[:, qt, DH : DH + 1]
                        )
                        nc.vector.tensor_scalar(
                            a16[:, qt, 1, :],
                            av3[:, qt, :DH],
                            rec[:, qt : qt + 1],
                            None,
                            mult,
                        )
                    nc.tensor.matmul(
                        tp,
                        a16[:, qt, :, :].rearrange("p a b -> p (a b)"),
                        id16[:],
                        start=True,
                        stop=True,
                    )
                    with nc.allow_low_precision(reason="bf16 attn"):
                        nc.vector.tensor_copy(
                            at_sb[
                                :,
                                1,
                                qc * 512 + qt * 128 : qc * 512 + qt * 128 + 128,
                            ],
                            tp,
                        )
                    out_proj(4 * (QC - 1) + qt)

    nc.compile()
    return nc


def shard_inputs(inputs):
    """Build the 8 per-core input maps: core c -> batch c//4, head-group c%4."""
    import ml_dtypes

    f8 = ml_dtypes.float8_e4m3
    bf = ml_dtypes.bfloat16

    x = np.asarray(inputs["x"], dtype=np.float32)
    Wq = np.asarray(inputs["Wq"], dtype=np.float32)
    Wk = np.asarray(inputs["Wk"], dtype=np.float32)
    Wv = np.asarray(inputs["Wv"], dtype=np.float32)
    bq = np.asarray(inputs["bq"], dtype=np.float32)
    bk = np.asarray(inputs["bk"], dtype=np.float32)
    bv = np.asarray(inputs["bv"], dtype=np.float32)
    Wo = np.asarray(inputs["Wo"], dtype=np.float32)

    def hilo(a, scale):
        s = (a * scale).astype(np.float32)
        hi = s.astype(f8)
        lo = (s - hi.astype(np.float32)).astype(f8)
        return hi, lo

    def xprep(xb):
        # [S, DIN] -> x^T [128, KT, S] -> DMA layout [QC, 128, KT, 512]
        xt = xb.T.reshape(KT, 128, QC, 512).transpose(2, 1, 0, 3)
        return np.ascontiguousarray(xt)

    def wprep(W, g):
        w = W[:, g * DQ : (g + 1) * DQ]  # [1024, 256]
        return np.ascontiguousarray(w.reshape(KT, 128, DQ).transpose(1, 0, 2))

    ident = np.eye(128, dtype=np.float32).astype(bf)

    in_maps = []
    for c in range(NCORES):
        b, g = divmod(c, HPC)
        xh, xl = hilo(xprep(x[b]), SX)
        m = {"xh": xh, "xl": xl, "ident": ident}
        for nm, W in (("wq", Wq), ("wk", Wk), ("wv", Wv)):
            hi, lo = hilo(wprep(W, g), SW)
            m[nm + "h"], m[nm + "l"] = hi, lo
        bqg = bq[g * DQ : (g + 1) * DQ] * SQ
        bkg = bk[g * DQ : (g + 1) * DQ] * SQ
        m["bq"] = np.ascontiguousarray(bqg.reshape(MT, 128).T)
        m["bk"] = np.ascontiguousarray(bkg.reshape(MT, 128).T)
        bvg = bv[g * DQ : (g + 1) * DQ].reshape(HPC, DH)
        m["bv"] = np.broadcast_to(bvg, (128, HPC, DH)).astype(bf)
        wog = Wo[g * DQ : (g + 1) * DQ, :]
        m["wo"] = (
            wog.reshape(MT, 128, DOUT).transpose(1, 0, 2).astype(bf)
        )
        m["wo"] = np.ascontiguousarray(m["wo"])
        in_maps.append(m)
    return in_maps


_PROGRAM_CACHE = []


def run_on_hw(inputs, trace=False):
    from concourse.bass_utils import run_bass_kernel_spmd

    if not _PROGRAM_CACHE:
        _PROGRAM_CACHE.append(build_program(1))
    nc = _PROGRAM_CACHE[0]
    in_maps = shard_inputs(inputs)
    res = run_bass_kernel_spmd(nc, in_maps, list(range(NCORES)), trace=False)
    bo = np.asarray(inputs["bo"], dtype=np.float32)
    out = np.zeros((B, S, DOUT), dtype=np.float32)
    for c in range(NCORES):
        out[c // HPC] += res.results[c]["out"]
    out += bo
    return out, res


def kernel(**inputs):
    out, _ = run_on_hw(inputs, trace=False)
    return out


# revision 11
# speedup vs baseline: 1.0788x; 1.0067x over previous
"""Multi-head attention kernel for Trainium2, sharded over 8 NeuronCores.

Problem: x[2,2048,1024] -> MHA(16 heads, dh=64) -> out[2,2048,512].

Sharding: core c handles batch b=c//4 and head-group g=c%4 (4 heads each).
Each core computes QKV, attention, and a partial output projection through
its 256-row slice of Wo; host sums the 4 head-group partials and adds bo.

Per-core design (engine budget: ScalarE exp stream is the wall at ~0.83ns
per score element; PE work is cut far below it with fp8 DoubleRow matmuls):
  - QKV projections in fp8 e4m3 DoubleRow (0.5 cyc/row, 2 k-tiles per
    instruction), 3-pass hi/lo error compensation (x*16 and W*256 scaled,
    split into e4m3 hi + e4m3 residual; hh+hl+lh passes, ll dropped).
    Host pre-quantizes, so splitting costs nothing on-chip.
  - Scores S^T[k,q] per head via one DoubleRow matmul per (head, k-tile,
    q-chunk): the pair dim carries Q-hi/Q-lo against 1-pass fp8 K
    (duplicated in SBUF), contraction dh=64. Q8/K8 = 32*(Q|K) quantized
    during the PSUM->SBUF bias-add copies.
  - exp on ScalarE with scale 1/8192 folded in (scores bounded, no max
    subtraction), bf16 output into a 32-slot SBUF ring.
  - AV in natural orientation: attn[q,65] += P^T-tile^T @ V_aug (V has a
    ones column -> row sums land in column 64). bf16, 65-cycle matmuls.
    All 4 q-subtile accumulation groups share one PSUM bank sequentially.
  - normalize: per-partition reciprocal multiply on the PSUM->SBUF copy;
    transpose attn via identity matmul into at^T for the output projection.
  - out partial [s,512] = at^T.T @ Wo (bf16) streamed out per s-tile.

Emission interleaves everything against the exp stream: scores for unit
(qc,h) + AV of the previous unit per k-pair, Q-projection and out-proj
fillers in fixed slots, so ScalarE never starves after the DMA lead-in.
"""

import sys

sys.path.insert(0, "/opt/trn_rl_repo")

import numpy as np
from contextlib import ExitStack

# Problem shapes (hardcoded per the harness contract).
B = 2
S = 2048
DIN = 1024
H = 16
DH = 64
DMODEL = H * DH  # 1024
DOUT = 512
NCORES = 8

# Per-core shard shapes.
HPC = 4  # heads per core
DQ = HPC * DH  # 256: per-core QKV width
KT = DIN // 128  # 8  k-tiles over d_in
MT = DQ // 128  # 2  m-tiles over per-core dq
QC = S // 512  # 4  q-chunks of 512
SKT = S // 128  # 16 seq k-tiles
VW = DH + 1  # 65: V columns per head incl. ones column
ESL = 32  # et ring slots

SX = 16.0  # x fp8 scale
SW = 256.0  # W fp8 scale
SQ = 32.0  # Q/K fp8 scale
PSC = 1.0 / (SX * SW)  # proj psum -> true value
QSC = SQ * PSC  # proj psum -> q8/k8 value (2^-7)
ESC = 1.0 / (SQ * SQ * np.sqrt(DH))  # score psum -> exp argument


def build_program(repeat=1):
    from concourse import bacc, tile
    import concourse.bass as bass
    import concourse.mybir as mybir

    f32 = mybir.dt.float32
    bf16 = mybir.dt.bfloat16
    f8 = mybir.dt.float8e4
    Exp = mybir.ActivationFunctionType.Exp
    DR = mybir.MatmulPerfMode.DoubleRow
    mult = mybir.AluOpType.mult
    add = mybir.AluOpType.add
    sub = mybir.AluOpType.subtract

    nc = bacc.Bacc("TRN2", target_bir_lowering=False, debug=False)

    xh_d = nc.dram_tensor("xh", [QC, 128, KT, 512], f8, kind="ExternalInput")
    xl_d = nc.dram_tensor("xl", [QC, 128, KT, 512], f8, kind="ExternalInput")
    wqh_d = nc.dram_tensor("wqh", [128, KT, DQ], f8, kind="ExternalInput")
    wql_d = nc.dram_tensor("wql", [128, KT, DQ], f8, kind="ExternalInput")
    wkh_d = nc.dram_tensor("wkh", [128, KT, DQ], f8, kind="ExternalInput")
    wkl_d = nc.dram_tensor("wkl", [128, KT, DQ], f8, kind="ExternalInput")
    wvh_d = nc.dram_tensor("wvh", [128, KT, DQ], f8, kind="ExternalInput")
    wvl_d = nc.dram_tensor("wvl", [128, KT, DQ], f8, kind="ExternalInput")
    bq_d = nc.dram_tensor("bq", [128, MT], f32, kind="ExternalInput")
    bk_d = nc.dram_tensor("bk", [128, MT], f32, kind="ExternalInput")
    bv_d = nc.dram_tensor("bv", [128, HPC, DH], bf16, kind="ExternalInput")
    wo_d = nc.dram_tensor("wo", [128, MT, DOUT], bf16, kind="ExternalInput")
    id_d = nc.dram_tensor("ident", [128, 128], bf16, kind="ExternalInput")
    out_d = nc.dram_tensor("out", [S, DOUT], f32, kind="ExternalOutput")

    with tile.TileContext(nc) as tc, ExitStack() as octx:
        consts = octx.enter_context(tc.tile_pool(name="consts", bufs=1))
        id16 = consts.tile([128, 128], bf16)
        bq32 = consts.tile([128, MT], f32)
        bk32 = consts.tile([128, MT], f32)
        bvb = consts.tile([128, HPC, DH], bf16)
        wo16 = consts.tile([128, MT, DOUT], bf16)
        nc.sync.dma_start(id16[:], id_d[:])

        for _rep in range(repeat):
            with ExitStack() as rctx:
                keep = rctx.enter_context(tc.tile_pool(name="keep", bufs=1))
                xh_sb = keep.tile([128, KT, S], f8)
                xl_sb = keep.tile([128, KT, S], f8)
                wqh = keep.tile([128, KT, DQ], f8)
                wql = keep.tile([128, KT, DQ], f8)
                wkh = keep.tile([128, KT, DQ], f8)
                wkl = keep.tile([128, KT, DQ], f8)
                wvh = keep.tile([128, KT, DQ], f8)
                wvl = keep.tile([128, KT, DQ], f8)
                # Q8/K8: head h=2m+j at partitions 64j..64j+64, m-tile m;
                # dim2 = (hi,lo) for Q8, duplicate slots for K8.
                q8_sb = keep.tile([128, MT, 2, S], f8)
                k8_sb = keep.tile([128, MT, 2, S], f8)
                v_sb = keep.tile([128, SKT, HPC, VW], bf16)
                et_sb = keep.tile([128, ESL, 512], bf16)
                at_sb = keep.tile([128, MT, S], bf16)
                nc.vector.memset(v_sb[:, :, :, DH], 1.0)

                sc_ps = rctx.enter_context(
                    tc.tile_pool(name="sc_ps", bufs=2, space="PSUM")
                )
                pj_ps = rctx.enter_context(
                    tc.tile_pool(name="pj_ps", bufs=2, space="PSUM")
                )
                av_ps = rctx.enter_context(
                    tc.tile_pool(name="av_ps", bufs=2, space="PSUM")
                )
                sm = rctx.enter_context(tc.tile_pool(name="sm", bufs=2))

                def dr12(ps, lhs_hl, rhs_hl):
                    """12 DoubleRow matmuls: 3-pass hi/lo over 4 k-tile pairs.

                    lhs_hl(sl, hi) / rhs_hl(sl, hi) -> stationary/moving
                    slices for k-tile pair sl; passes hh + hl + lh."""
                    for tp in range(KT // 2):
                        sl = slice(2 * tp, 2 * tp + 2)
                        first, last = tp == 0, tp == KT // 2 - 1
                        nc.tensor.matmul(
                            ps, lhs_hl(sl, True), rhs_hl(sl, True),
                            start=first, stop=False, perf_mode=DR,
                        )
                        nc.tensor.matmul(
                            ps, lhs_hl(sl, True), rhs_hl(sl, False),
                            start=False, stop=False, perf_mode=DR,
                        )
                        nc.tensor.matmul(
                            ps, lhs_hl(sl, False), rhs_hl(sl, True),
                            start=False, stop=last, perf_mode=DR,
                        )

                def qk_proj(wh, wl, m, qc, is_q):
                    """Q^T/K^T m-tile for q-chunk qc -> q8/k8 (scaled fp8)."""
                    qsl = slice(qc * 512, (qc + 1) * 512)
                    msl = slice(m * 128, (m + 1) * 128)
                    ps = pj_ps.tile([128, 512], f32, tag="pj", name="ps")
                    dr12(
                        ps[:],
                        lambda sl, hi: (wh if hi else wl)[:, sl, msl],
                        lambda sl, hi: (xh_sb if hi else xl_sb)[:, sl, qsl],
                    )
                    with nc.allow_low_precision(reason="fp8 by design"):
                        if is_q:
                            q16 = sm.tile([128, 512], bf16, tag="q16")
                            nc.vector.tensor_scalar(
                                q16[:], ps[:], QSC, bq32[:, m : m + 1], mult, add
                            )
                            nc.vector.tensor_copy(
                                q8_sb[:, m, 0, qsl], q16[:]
                            )
                            nc.vector.tensor_tensor(
                                q8_sb[:, m, 1, qsl],
                                q16[:],
                                q8_sb[:, m, 0, qsl],
                                sub,
                            )
                        else:
                            nc.vector.tensor_scalar(
                                k8_sb[:, m, 0, qsl],
                                ps[:],
                                QSC,
                                bk32[:, m : m + 1],
                                mult,
                                add,
                            )
                            nc.vector.tensor_copy(
                                k8_sb[:, m, 1, qsl], k8_sb[:, m, 0, qsl]
                            )

                def v_proj(st):
                    """Natural-orientation V s-tile st -> v_sb (bf16 + bias)."""
                    ssl = slice(st * 128, (st + 1) * 128)
                    ps = pj_ps.tile([128, 512], f32, tag="pj", name="ps")
                    dr12(
                        ps[:, :DQ],
                        lambda sl, hi: (xh_sb if hi else xl_sb)[:, sl, ssl],
                        lambda sl, hi: (wvh if hi else wvl)[:, sl, :],
                    )
                    with nc.allow_low_precision(reason="bf16 by design"):
                        nc.vector.scalar_tensor_tensor(
                            v_sb[:, st, :, :DH],
                            ps[:, :DQ].rearrange("p (h d) -> p h d", h=HPC),
                            PSC,
                            bvb[:],
                            mult,
                            add,
                        )

                def scores_unit(qc, h, u):
                    """k-tiles 2u,2u+1 of S^T for (qc,h): 2 DR matmuls + exp."""
                    j, m = h % 2, h // 2
                    base = slice(64 * j, 64 * j + 64)
                    qsl = slice(qc * 512, (qc + 1) * 512)
                    sc = sc_ps.tile([128, 2, 512], f32, tag="sc", name="sc")
                    for i in range(2):
                        kt = 2 * u + i
                        nc.tensor.matmul(
                            sc[:, i, :],
                            k8_sb[base, m, :, kt * 128 : (kt + 1) * 128],
                            q8_sb[base, m, :, qsl],
                            start=True,
                            stop=True,
                            perf_mode=DR,
                        )
                    slot = ((qc * HPC + h) * SKT + 2 * u) % ESL
                    with nc.allow_low_precision(reason="bf16 probs by design"):
                        nc.scalar.activation(
                            et_sb[:, slot : slot + 2, :], sc[:], Exp, scale=ESC
                        )

                def av_slice(qc, h, av3, qt, k0):
                    """8 k-tiles of the attn accumulation for q-subtile qt."""
                    ubase = (qc * HPC + h) * SKT
                    qts = slice(qt * 128, (qt + 1) * 128)
                    for kt in range(k0, k0 + 8):
                        nc.tensor.matmul(
                            av3[:, qt, :],
                            et_sb[:, (ubase + kt) % ESL, qts],
                            v_sb[:, kt, h, :],
                            start=(kt == 0),
                            stop=(kt == SKT - 1),
                        )

                def finish(qc, h, av, av3, a16):
                    """Normalize closed attn accums; transpose on pair end."""
                    j = h % 2
                    rec = sm.tile([128, HPC], f32, tag="rec")
                    with nc.allow_low_precision(reason="recip of ~2e3 sums"):
                        nc.vector.reciprocal(rec[:], av3[:, :, DH])
                        for qt in range(4):
                            nc.vector.tensor_scalar(
                                a16[:, qt, j, :],
                                av3[:, qt, :DH],
                                rec[:, qt : qt + 1],
                                None,
                                mult,
                            )
                    if j == 1:
                        p = h // 2
                        tp = av[:, HPC * VW : HPC * VW + 128]
                        for qt in range(4):
                            nc.tensor.matmul(
                                tp,
                                a16[:, qt, :, :].rearrange("p a b -> p (a b)"),
                                id16[:],
                                start=True,
                                stop=True,
                            )
                            with nc.allow_low_precision(reason="bf16 attn"):
                                nc.vector.tensor_copy(
                                    at_sb[
                                        :,
                                        p,
                                        qc * 512 + qt * 128 : qc * 512 + qt * 128 + 128,
                                    ],
                                    tp,
                                )

                def out_proj(m):
                    """Output partial for s-tile m."""
                    ps = pj_ps.tile([128, 512], f32, tag="pj", name="ps")
                    for k2 in range(MT):
                        nc.tensor.matmul(
                            ps[:],
                            at_sb[:, k2, m * 128 : (m + 1) * 128],
                            wo16[:, k2, :],
                            start=(k2 == 0),
                            stop=(k2 == MT - 1),
                        )
                    ot = sm.tile([128, DOUT], f32, tag="ot")
                    nc.vector.tensor_copy(ot[:], ps[:])
                    nc.sync.dma_start(out_d[m * 128 : (m + 1) * 128, :], ot[:])

                # ---- Lead-in. DMA issue is serialized per engine, so x-hi
                # chunks go out on the idle Pool engine while SP carries x-lo
                # interleaved with weights; late-needed consts trail. PE
                # warms its pstate on the identity, then Q-projection first
                # (it heads the longest DVE chain), K, and the (0,0) score
                # units per chunk. V and m1 projections defer to fillers.
                nc.sync.dma_start(wqh[:], wqh_d[:])
                nc.sync.dma_start(wql[:], wql_d[:])
                nc.gpsimd.dma_start(xh_sb[:, :, 0:512], xh_d[0])
                nc.gpsimd.dma_start(bq32[:], bq_d[:])
                nc.gpsimd.dma_start(bk32[:], bk_d[:])
                nc.gpsimd.dma_start(wkh[:], wkh_d[:])
                nc.gpsimd.dma_start(wkl[:], wkl_d[:])
                nc.sync.dma_start(xl_sb[:, :, 0:512], xl_d[0])
                for kc in range(1, QC):
                    csl = slice(kc * 512, (kc + 1) * 512)
                    nc.gpsimd.dma_start(xh_sb[:, :, csl], xh_d[kc])
                    nc.sync.dma_start(xl_sb[:, :, csl], xl_d[kc])
                    if kc == 1:
                        nc.sync.dma_start(wvh[:], wvh_d[:])
                        nc.sync.dma_start(wvl[:], wvl_d[:])
                nc.sync.dma_start(bvb[:], bv_d[:])
                nc.sync.dma_start(wo16[:], wo_d[:])
                wm = sc_ps.tile([128, 2, 512], f32, tag="sc", name="wm")
                for _ in range(28):
                    nc.tensor.matmul(
                        wm[:, 0, :128], id16[:], id16[:], start=True, stop=True
                    )
                qk_proj(wqh, wql, 0, 0, is_q=True)
                for kc in range(QC):
                    qk_proj(wkh, wkl, 0, kc, is_q=False)
                    scores_unit(0, 0, 2 * kc)
                    scores_unit(0, 0, 2 * kc + 1)

                # ---- Main pipeline over (qc, h) units.
                units = [(qc, h) for qc in range(QC) for h in range(HPC)]
                prev = None
                for qc, h in units:
                    av = av_ps.tile([128, HPC * VW + 128], f32, tag="av", name="av")
                    av3 = av[:, : HPC * VW].rearrange("p (t c) -> p t c", t=HPC)
                    if h % 2 == 0:
                        a16 = sm.tile([128, 4, 2, DH], bf16, tag="a16", bufs=2)
                    cur = (qc, h, av, av3, a16)
                    for u in range(8):
                        if (qc, h) != (0, 0):
                            scores_unit(qc, h, u)
                        if prev is not None:
                            av_slice(prev[0], prev[1], prev[3], u // 2, 8 * (u % 2))
                        if (qc, h) == (0, 0):
                            for st in (2 * u, 2 * u + 1):
                                if st < 14:
                                    v_proj(st)
                        if (qc, h) == (0, 1):
                            if u == 0:
                                v_proj(14)
                                v_proj(15)
                            if u % 2 == 1:
                                qk_proj(wkh, wkl, 1, u // 2, is_q=False)
                            if u == 6:
                                qk_proj(wqh, wql, 1, 0, is_q=True)
                        if u == 2 and h >= 2 and qc < QC - 1:
                            qk_proj(wqh, wql, h - 2, qc + 1, is_q=True)
                        if qc >= 1 and u == 4 and h >= 1:
                            out_proj(4 * (qc - 1) + h - 1)
                        if qc >= 1 and u == 6 and h == 3:
                            out_proj(4 * (qc - 1) + 3)
                    if prev is not None:
                        finish(prev[0], prev[1], prev[2], prev[3], prev[4])
                    prev = cur

                # ---- Tail: close the last unit (3,3) pipelined per
                # q-subtile: out-proj m-tile 12+qt only needs subtile qt.
                # Odd subtiles accumulate in a second av bank so normalize
                # of one subtile overlaps accumulation of the next.
                qc, h, av, av3, a16 = prev
                avb = av_ps.tile([128, HPC * VW + 128], f32, tag="av", name="avb")
                av3b = avb[:, : HPC * VW].rearrange("p (t c) -> p t c", t=HPC)
                rec = sm.tile([128, HPC], f32, tag="rec")
                for qt in range(4):
                    a3 = av3 if qt % 2 == 0 else av3b
                    av_slice(qc, h, a3, qt, 0)
                    av_slice(qc, h, a3, qt, 8)
                for qt in range(4):
                    a3 = av3 if qt % 2 == 0 else av3b
                    with nc.allow_low_precision(reason="recip of ~2e3 sums"):
                        nc.vector.reciprocal(
                            rec[:, qt : qt + 1], a3[:, qt, DH : DH + 1]
                        )
                        nc.vector.tensor_scalar(
                            a16[:, qt, 1, :],
                            a3[:, qt, :DH],
                            rec[:, qt : qt + 1],
                            None,
                            mult,
                        )
                    tpt = pj_ps.tile([128, 512], f32, tag="pj", name="tpt")
                    nc.tensor.matmul(
                        tpt[:, :128],
                        a16[:, qt, :, :].rearrange("p a b -> p (a b)"),
                        id16[:],
                        start=True,
                        stop=True,
                    )
                    with nc.allow_low_precision(reason="bf16 attn"):
                        nc.vector.tensor_copy(
                            at_sb[
                                :,
                                1,
                                qc * 512 + qt * 128 : qc * 512 + qt * 128 + 128,
                            ],
                            tpt[:, :128],
                        )
                    out_proj(4 * (QC - 1) + qt)

    nc.compile()
    return nc


def shard_inputs(inputs):
    """Build the 8 per-core input maps: core c -> batch c//4, head-group c%4."""
    import ml_dtypes

    f8 = ml_dtypes.float8_e4m3
    bf = ml_dtypes.bfloat16

    x = np.asarray(inputs["x"], dtype=np.float32)
    Wq = np.asarray(inputs["Wq"], dtype=np.float32)
    Wk = np.asarray(inputs["Wk"], dtype=np.float32)
    Wv = np.asarray(inputs["Wv"], dtype=np.float32)
    bq = np.asarray(inputs["bq"], dtype=np.float32)
    bk = np.asarray(inputs["bk"], dtype=np.float32)
    bv = np.asarray(inputs["bv"], dtype=np.float32)
    Wo = np.asarray(inputs["Wo"], dtype=np.float32)

    def hilo(a, scale):
        s = (a * scale).astype(np.float32)
        hi = s.astype(f8)
        lo = (s - hi.astype(np.float32)).astype(f8)
        return hi, lo

    def xprep(xb):
        # [S, DIN] -> x^T [128, KT, S] -> DMA layout [QC, 128, KT, 512]
        xt = xb.T.reshape(KT, 128, QC, 512).transpose(2, 1, 0, 3)
        return np.ascontiguousarray(xt)

    def wprep(W, g):
        w = W[:, g * DQ : (g + 1) * DQ]  # [1024, 256]
        return np.ascontiguousarray(w.reshape(KT, 128, DQ).transpose(1, 0, 2))

    ident = np.eye(128, dtype=np.float32).astype(bf)

    in_maps = []
    for c in range(NCORES):
        b, g = divmod(c, HPC)
        xh, xl = hilo(xprep(x[b]), SX)
        m = {"xh": xh, "xl": xl, "ident": ident}
        for nm, W in (("wq", Wq), ("wk", Wk), ("wv", Wv)):
            hi, lo = hilo(wprep(W, g), SW)
            m[nm + "h"], m[nm + "l"] = hi, lo
        bqg = bq[g * DQ : (g + 1) * DQ] * SQ
        bkg = bk[g * DQ : (g + 1) * DQ] * SQ
        m["bq"] = np.ascontiguousarray(bqg.reshape(MT, 128).T)
        m["bk"] = np.ascontiguousarray(bkg.reshape(MT, 128).T)
        bvg = bv[g * DQ : (g + 1) * DQ].reshape(HPC, DH)
        m["bv"] = np.broadcast_to(bvg, (128, HPC, DH)).astype(bf)
        wog = Wo[g * DQ : (g + 1) * DQ, :]
        m["wo"] = (
            wog.reshape(MT, 128, DOUT).transpose(1, 0, 2).astype(bf)
        )
        m["wo"] = np.ascontiguousarray(m["wo"])
        in_maps.append(m)
    return in_maps


_PROGRAM_CACHE = []


def run_on_hw(inputs, trace=False):
    from concourse.bass_utils import run_bass_kernel_spmd

    if not _PROGRAM_CACHE:
        _PROGRAM_CACHE.append(build_program(1))
    nc = _PROGRAM_CACHE[0]
    in_maps = shard_inputs(inputs)
    res = run_bass_kernel_spmd(nc, in_maps, list(range(NCORES)), trace=False)
    bo = np.asarray(inputs["bo"], dtype=np.float32)
    out = np.zeros((B, S, DOUT), dtype=np.float32)
    for c in range(NCORES):
        out[c // HPC] += res.results[c]["out"]
    out += bo
    return out, res


def kernel(**inputs):
    out, _ = run_on_hw(inputs, trace=False)
    return out
